# revision 2
# baseline (speedup 1.0000x reference)
"""Trainium2 Bass kernel for nn_FMNet pixel-shuffle + sigmoid.

reference:  x = FV[:, 64:, :, :]                                 # [B, 64, 64, 64]
            out[b, 8i+r, 8j+c'] = sigmoid(x[b, 8r+c', i, j])     # [B, 1, 512, 512]

Per core (4 batches), partition p = 4*i2 + b (i2-major, batch-minor ->
every per-batch DMA spreads over the full 128-partition range with
stride 4, which engages ~1.3-1.7x more SDMA lanes than contiguous-32p):
  tin  free = (c:64, ip:2, j:64)        x[b, 64+c, 2*i2+ip, j]
  tout free = (ip:2, r:8, j:64, c':8)   out rows 16*i2+8*ip+r, col 8j+c'
  tp PSUM [128, 4096] = 4 octant slots of 1024, slot free = (ip, j, c')

Measured HW facts (exp1-exp3 + v5 traces on this device):
  - The 512-B-chunk load ceiling is AGGREGATE ~250-285 GB/s no matter how
    many queues carry it; disjoint partition ranges (sync: b0/b1, SWDGE:
    b2/b3) give the best aggregate (285), so loads use 2 queues.
  - Interleaved (stride-8) access costs 2.3x on ACT/DVE against SBUF but
    is FREE against PSUM: sigmoid runs ACT contig-read -> PSUM
    strided-write (1.12us/octant), DVE evicts PSUM -> tout contig
    (1.5us/octant), pipelined ring of 4 PSUM slots. Last octant is a
    direct strided ACT (2.1us) so nothing trails it but the store issue.
  - Stores: two whole-half 128-p DMAs (8-KiB chunks); transfers mostly
    ride the fixed ~8us walrus epilogue (nothing in the NEFF waits on
    them; the runtime quiesces queues before the host reads OUT).
  - ACT sigmoid table load (~1.7us ScalarE) runs during the load wait.
"""

import os
import sys

if "/opt/trn_rl_repo" not in sys.path:
    sys.path.insert(0, "/opt/trn_rl_repo")

import numpy as np

import concourse.bass as bass
from concourse import mybir
from concourse.bass_utils import run_bass_kernel_spmd

N_CORES = 8
B = 32
B_LOC = B // N_CORES   # 4
H = W = 512
S = 64

F32 = mybir.dt.float32
SIG = mybir.ActivationFunctionType.Sigmoid

LAST_EXEC_NS = None
_cached_nc = None

# Load plan: (b, c0, nch) per engine, issue order; p = 4*i2 + b stride-4.
LOADS = {
    "sync": [(0, 0, 16), (3, 0, 16), (0, 16, 16), (3, 32, 16), (0, 48, 8),
             (3, 48, 8), (0, 56, 8), (3, 56, 8)],
    "gpsimd": [(1, 0, 16), (1, 16, 16), (3, 16, 16), (1, 32, 16),
               (1, 48, 8), (1, 56, 8)],
    "scalar": [(2, 0, 16), (2, 16, 16), (2, 32, 16), (0, 32, 16),
               (2, 48, 8), (2, 56, 8)],
}

# ACT stages: ("o", octant, slot) via PSUM; ("d", octant) direct to tout.
ACTS = [("o", 0, 0), ("o", 1, 1), ("o", 2, 2), ("o", 3, 3),
        ("o", 4, 0), ("o", 5, 1), ("o", 6, 2), ("d", 7)]
# EVs: octant k evicted from slot k%4 after sem_act >= k+1.
EVS = [0, 1, 2, 3, 4, 5, 6]
ACT_EV_WAIT = {4: 1, 5: 2, 6: 3}   # slot reuse: ACT idx -> sem_ev threshold
# store h: 0 -> rows 0..3 (ev>=4), 1 -> rows 4..7 (ev>=7 & act>=8)
STORE_GATE = {0: (4, None), 1: (7, 8)}
# per-batch stores (b, h) per engine; scalar takes h1 tails (free after ACTs)
STORES = {"sync": [(0, 0), (1, 0), (2, 1), (3, 1)],
          "gpsimd": [(2, 0), (3, 0), (1, 1)], "scalar": [(0, 1)]}


WAVE_OF_C0 = {0: 0, 16: 1, 32: 2, 48: 3, 56: 4}
N_WAVES = 5
OCT_WAVE = [0, 0, 1, 1, 2, 2, 3, 4]   # octant -> wave whose data it needs


def _wave_counts():
    """(wave, engine) -> piece count (sem threshold = 16 * count).
    DMA completion order is NOT issue order, so each wave gets its own
    semaphore per engine and gates count exactly that wave's pieces."""
    counts = {}
    for e, lst in LOADS.items():
        for b, c0, nch in lst:
            w = WAVE_OF_C0[c0]
            counts[(w, e)] = counts.get((w, e), 0) + 1
    return counts


def _install_trace_hook():
    try:
        import types

        import antenv

        try:
            from antenv.axon_hooks import get_axon_ntff_profile_hook  # noqa: F401

            return
        except ImportError:
            pass
        mod = types.ModuleType("antenv.axon_hooks")
        _state = {"hook": None}
        mod.set_axon_ntff_profile_hook = lambda h: _state.__setitem__("hook", h)
        mod.get_axon_ntff_profile_hook = lambda: _state["hook"]
        sys.modules["antenv.axon_hooks"] = mod
        antenv.axon_hooks = mod
        from trn_agent_boot.trn_boot import _ntff_profile_via_ctypes

        mod.set_axon_ntff_profile_hook(
            _ntff_profile_via_ctypes("/opt/axon/libaxon_pjrt.so")
        )
    except Exception:
        pass


def _build_nc():
    import contextlib

    nc = bass.Bass("TRN2", num_devices=N_CORES)
    FV = nc.declare_dram_parameter("FV", [B_LOC, 128, S, S], F32, isOutput=False)
    OUT = nc.declare_dram_parameter("OUT", [B_LOC, W, H], F32, isOutput=True)

    tin = nc.alloc_sbuf_tensor("tin", [128, 8192], F32)
    tout = nc.alloc_sbuf_tensor("tout", [128, 8192], F32)
    tp = nc.alloc_psum_tensor("tp", [128, 4096], F32)
    scratch = nc.alloc_sbuf_tensor("scratch", [1, 8], F32)

    fv = FV[:]
    out = OUT[:]
    wave_counts = _wave_counts()

    def load_aps(b, c0, nch):
        src = fv[b, 64 + c0 : 64 + c0 + nch].rearrange(
            "c (i2 ip) j -> i2 c (ip j)", ip=2
        )
        dst = tin.ap()[b:128:4, 128 * c0 : 128 * (c0 + nch)].rearrange(
            "p (c v) -> p c v", c=nch
        )
        return dst, src

    def act_aps(st):
        if st[0] == "o":
            _, o, s = st
            in_v = tin.ap()[:, 1024 * o : 1024 * (o + 1)].rearrange(
                "p (c ip j) -> p c ip j", c=8, ip=2
            )
            out_v = tp.ap()[:, 1024 * s : 1024 * (s + 1)].rearrange(
                "p (ip j c) -> p c ip j", ip=2, j=64
            )
        else:
            _, o = st
            in_v = tin.ap()[:, 1024 * o : 1024 * (o + 1)].rearrange(
                "p (c ip j) -> p c ip j", c=8, ip=2
            )
            out_v = tout.ap().rearrange(
                "p (ip r j c) -> p r c ip j", ip=2, r=8, j=64
            )[:, o]
        return out_v, in_v

    def ev_aps(o):
        s = o % 4
        in_v = tp.ap()[:, 1024 * s : 1024 * (s + 1)].rearrange(
            "p (v one) -> p v one", one=1
        )
        out_v = tout.ap().rearrange("p (ip r v) -> p r ip v", ip=2, r=8)[:, o]
        return out_v, in_v

    def store_aps(b, h):
        dst = out[b].rearrange(
            "(i2 ip rh rr) Q -> i2 ip rh (rr Q)", i2=32, ip=2, rh=2
        )[:, :, h, :]
        src = tout.ap()[b:128:4, :].rearrange(
            "p (ip rh v) -> p ip rh v", ip=2, rh=2
        )[:, :, h]
        return dst, src

    with contextlib.ExitStack() as stack:
        block = stack.enter_context(nc.Block(no_gpsimd_drain=True))
        sem_w = {
            k: stack.enter_context(nc.semaphore(f"sem_w{k[0]}_{k[1]}"))
            for k in wave_counts
        }
        sem_act = stack.enter_context(nc.semaphore("sem_act"))
        sem_ev = stack.enter_context(nc.semaphore("sem_ev"))
        sem_o = {
            e: stack.enter_context(nc.semaphore(f"sem_o_{e}")) for e in STORES
        }

        def emit_stores(eng, name):
            for b, h in STORES[name]:
                ev_th, act_th = STORE_GATE[h]
                eng.wait_ge(sem_ev, ev_th)
                if act_th is not None:
                    eng.wait_ge(sem_act, act_th)
                dst, src = store_aps(b, h)
                eng.dma_start(out=dst, in_=src).then_inc(sem_o[name], 16)

        def emit_loads(eng, name):
            for b, c0, nch in LOADS[name]:
                dst, src = load_aps(b, c0, nch)
                eng.dma_start(out=dst, in_=src).then_inc(
                    sem_w[(WAVE_OF_C0[c0], name)], 16
                )

        @block.sync
        def _(sync: bass.BassEngine):
            emit_loads(sync, "sync")
            emit_stores(sync, "sync")

        @block.gpsimd
        def _(g: bass.BassEngine):
            emit_loads(g, "gpsimd")
            emit_stores(g, "gpsimd")

        @block.scalar
        def _(scalar: bass.BassEngine):
            emit_loads(scalar, "scalar")
            # dummy op pulls ACT_TABLE_LOAD (sigmoid) off the critical path
            scalar.activation(
                scratch.ap()[0:1, 0:1],
                nc.const_aps.tensor(0.0, (1, 1), F32),
                SIG,
            )
            waited = set()
            for idx, st in enumerate(ACTS):
                w = OCT_WAVE[st[1]]
                for e in ("sync", "gpsimd", "scalar"):
                    k = (w, e)
                    if k in wave_counts and k not in waited:
                        scalar.wait_ge(sem_w[k], 16 * wave_counts[k])
                        waited.add(k)
                if idx in ACT_EV_WAIT:
                    scalar.wait_ge(sem_ev, ACT_EV_WAIT[idx])
                out_v, in_v = act_aps(st)
                scalar.activation(out_v, in_v, SIG).then_inc(sem_act, 1)
            emit_stores(scalar, "scalar")

        @block.vector
        def _(vec: bass.BassEngine):
            for o in EVS:
                vec.wait_ge(sem_act, o + 1)
                out_v, in_v = ev_aps(o)
                vec.tensor_reduce(
                    out_v, in_v, mybir.AxisListType.X, mybir.AluOpType.max
                ).then_inc(sem_ev, 1)

    return nc


def kernel(FV, batch_size=None, W=None, H=None, **_ignored):
    global _cached_nc, LAST_EXEC_NS
    FV = np.asarray(FV, dtype=np.float32)
    assert FV.shape == (B, 128, S, S), FV.shape

    trace = bool(os.environ.get("BASS_TRACE"))
    if trace:
        _install_trace_hook()

    if _cached_nc is None:
        _cached_nc = _build_nc()
    nc = _cached_nc

    in_maps = [{"FV": FV[k * B_LOC : (k + 1) * B_LOC]} for k in range(N_CORES)]
    res = None
    for attempt in range(3):
        try:
            res = run_bass_kernel_spmd(nc, in_maps, list(range(N_CORES)), trace=trace)
            break
        except Exception:
            if attempt == 2:
                raise
            import time

            time.sleep(2.0)
    if trace:
        LAST_EXEC_NS = res.exec_time_ns

    outs = [res.results[k]["OUT"] for k in range(N_CORES)]
    full = np.concatenate(outs, axis=0)  # [32, 512, 512]
    return full[:, None, :, :].astype(np.float32)
